# revision 7
# baseline (speedup 1.0000x reference)
"""Trainium2 Bass kernel for nn_CrossAttention (sparse_attention), v2.

Data-parallel over batch N=8 across the 8 NeuronCores; each core computes one
batch element's full attention independently (no collectives).

Restructure vs v1 (403us):
  - AV computed TRANSPOSED: zT[d, q] accumulated in PSUM with vN (values in
    normal [l, d] layout, ones column appended) as the stationary matmul
    operand and expT / SbT streamed in 128-col chunks with clean per-region
    start/stop accumulation chains. Kills ~88 tiny matmuls + 64 PE transposes
    + 64 per-tile DVE normalize chains per head, and zT lands directly in the
    orientation the output projection consumes.
  - Softmax denominators (ones-column) land in zpT row 64; normalization via
    reciprocal + partition-broadcast DMA of 1/Z + two DVE tensor_tensor ops.
    The fully-masked last row (uniform 1/S) is restored exactly by overwriting
    zpT[:, 1023] with colsum(vals) and Z=S before normalizing.
  - Skew: ONE dma_start_transpose per head. brd DRAM uses row stride 1151
    with qi-block stride 128*1150 so the skew diagonal over all 8 q-tiles is
    a single linear 2D AP [[1150, 1024], [1, 1024]]; the XBAR fold writes
    SbT[jp, jb, q] directly in per-j-block layout. Blocks jb > qi hold junk
    that is transferred but never consumed.
  - Software pipelined: zsT + combine for head h-1 are emitted after zpT of
    head h so the DRAM bounce latency hides under the next head's compute.
  - Output written bf16, cast to f32 on host.
"""
import os

os.environ.setdefault("MYCRO_LOCAL_CACHE", "1")

import numpy as np
import ml_dtypes

import concourse.bass as bass
import concourse.mybir as mybir
import concourse.tile as tile
from concourse import bacc
from concourse.bass_utils import run_bass_kernel_spmd

N, S, EMB, H, D = 8, 1024, 512, 8, 64
DA = D + 1              # augmented (ones column)
NCORES = 8
BF = mybir.dt.bfloat16
F32 = mybir.dt.float32
bf16 = ml_dtypes.bfloat16
QT = S // 128           # 8 tiles of 128
L = 1151                # brd row length (elements)
QSTRIDE = 128 * (L - 1)  # qi-block stride in the overlap layout


def build_nc(heads=H):
    nc = bacc.Bacc("TRN2", target_bir_lowering=False, debug=False,
                   num_devices=NCORES)

    # ---- DRAM I/O ----
    vkq = nc.dram_tensor("vkq", [H, DA, 3, S], BF, kind="ExternalInput")
    WvTa = nc.dram_tensor("WvTa", [DA, D], BF, kind="ExternalInput")
    WkTa = nc.dram_tensor("WkTa", [DA, D], BF, kind="ExternalInput")
    WqTa = nc.dram_tensor("WqTa", [DA, D], BF, kind="ExternalInput")
    WvN = nc.dram_tensor("WvN", [DA, DA], BF, kind="ExternalInput")
    ETa = nc.dram_tensor("ETa", [D, S], BF, kind="ExternalInput")
    WoT = nc.dram_tensor("WoT", [EMB, EMB], BF, kind="ExternalInput")
    boC = nc.dram_tensor("boC", [4, 128, 1], F32, kind="ExternalInput")
    M01 = nc.dram_tensor("M01", [128, 128], BF, kind="ExternalInput")
    outT = nc.dram_tensor("outT", [EMB, S], BF, kind="ExternalOutput")

    from contextlib import ExitStack
    with tile.TileContext(nc) as tc, ExitStack() as ctx:
        consts = ctx.enter_context(tc.tile_pool(name="consts", bufs=1))
        inp = ctx.enter_context(tc.tile_pool(name="inp", bufs=2))
        proj = ctx.enter_context(tc.tile_pool(name="proj", bufs=2))
        vnp = ctx.enter_context(tc.tile_pool(name="vnp", bufs=2))
        expp = ctx.enter_context(tc.tile_pool(name="expp", bufs=2))
        brp = ctx.enter_context(tc.tile_pool(name="brp", bufs=2))
        stp = ctx.enter_context(tc.tile_pool(name="stp", bufs=2))
        small = ctx.enter_context(tc.tile_pool(name="small", bufs=2))
        tmpp = ctx.enter_context(tc.tile_pool(name="tmpp", bufs=2))
        binvp = ctx.enter_context(tc.tile_pool(name="binvp", bufs=2))
        outp = ctx.enter_context(tc.tile_pool(name="outp", bufs=2))
        ztp = ctx.enter_context(tc.tile_pool(name="ztp", bufs=1))
        ps_mm = ctx.enter_context(tc.tile_pool(name="ps_mm", bufs=3,
                                               space="PSUM"))
        ps_zp0 = ctx.enter_context(tc.tile_pool(name="ps_zp0", bufs=1,
                                                space="PSUM"))
        ps_zp1 = ctx.enter_context(tc.tile_pool(name="ps_zp1", bufs=1,
                                                space="PSUM"))
        ps_zs0 = ctx.enter_context(tc.tile_pool(name="ps_zs0", bufs=1,
                                                space="PSUM"))
        ps_zs1 = ctx.enter_context(tc.tile_pool(name="ps_zs1", bufs=1,
                                                space="PSUM"))
        dram = ctx.enter_context(tc.tile_pool(name="dram", bufs=2,
                                              space="DRAM"))

        # ---- constants ----
        c_wv = consts.tile([DA, D], BF)
        c_wk = consts.tile([DA, D], BF)
        c_wq = consts.tile([DA, D], BF)
        c_wvn = consts.tile([DA, DA], BF)
        c_et = consts.tile([D, S], BF)
        c_wo = consts.tile([128, 4, EMB], BF)   # [e-part, e-chunk, e_out]
        c_bo = consts.tile([128, 4, 1], F32)
        c_m01 = consts.tile([128, 128], BF)
        nc.sync.dma_start(out=c_wv[:], in_=WvTa[:])
        nc.sync.dma_start(out=c_wk[:], in_=WkTa[:])
        nc.sync.dma_start(out=c_wq[:], in_=WqTa[:])
        nc.sync.dma_start(out=c_wvn[:], in_=WvN[:])
        nc.sync.dma_start(out=c_et[:], in_=ETa[:])
        nc.sync.dma_start(
            out=c_wo[:], in_=WoT[:].rearrange("(c p) e -> p c e", p=128))
        nc.sync.dma_start(
            out=c_bo[:], in_=boC[:].rearrange("c p one -> p c one"))
        nc.sync.dma_start(out=c_m01[:], in_=M01[:])

        # zT chunks: [128 (= 2 heads of d), S] bf16, 4 of them
        zTc = [ztp.tile([128, S], BF, tag=f"ztc{i}", name=f"ztc{i}")
               for i in range(4)]

        def emit_combine_mul(pv):
            # tmp = zpT[0:64, :] * (1/Z broadcast)
            tmp = tmpp.tile([D, S], BF, tag="tmp")
            for g in range(2):
                nc.vector.tensor_mul(tmp[:, bass.ts(g, 512)],
                                     pv["zp"][g][:D, :],
                                     pv["binv"][:, bass.ts(g, 512)])
            pv["tmp"] = tmp

        def emit_zsT(pv):
            zs = [ps_zs0.tile([D, 512], F32, tag="zs0", name="zs0"),
                  ps_zs1.tile([D, 512], F32, tag="zs1", name="zs1")]
            vn, st = pv["vn"], pv["st"]
            # one start per PSUM bank: start lazily zeroes the WHOLE 2KB zero
            # region, so the first matmul touching each bank starts the group
            # and every other one accumulates (unwritten cols read as zero).
            for lb in range(QT):
                for qb in range(lb, QT):
                    g, coff = qb // 4, (qb % 4) * 128
                    nc.tensor.matmul(
                        zs[g][:, coff:coff + 128],
                        vn[:, lb, 0:D],
                        st[:, lb, bass.ts(qb, 128)],
                        start=(lb == 0 and qb in (0, 4)),
                        stop=((lb == 3 and qb == 3) or
                              (lb == 7 and qb == 7)))
            pv["zs"] = zs

        def emit_combine_add(pv):
            ph = pv["h"]
            dst = zTc[ph // 2]
            r0 = (ph % 2) * D
            for g in range(2):
                nc.vector.tensor_add(dst[r0:r0 + D, bass.ts(g, 512)],
                                     pv["tmp"][:, bass.ts(g, 512)],
                                     pv["zs"][g][:, :])

        prev = None
        for h in range(heads):
            cur = {"h": h}
            # ---- 1. staged input [DA, 3, S]: slot 0=v, 1=k, 2=q ----
            xin = inp.tile([DA, 3, S], BF, tag="xin")
            nc.gpsimd.dma_start(out=xin[:], in_=vkq[h])

            # ---- 2. transposed projections kT/qT/vT [64, S] ----
            kT = proj.tile([D, S], BF, tag="kT")
            qT = proj.tile([D, S], BF, tag="qT")
            vT = proj.tile([D, S], BF, tag="vT")
            for (dst, w, xi) in ((kT, c_wk, 1), (qT, c_wq, 2), (vT, c_wv, 0)):
                for sl in range(2):
                    cols = bass.ts(sl, 512)
                    pm = ps_mm.tile([128, 512], F32, tag="mm")
                    nc.tensor.matmul(pm[:D, :], w[:], xin[:, xi, cols],
                                     start=True, stop=True)
                    nc.scalar.copy(out=dst[:, cols], in_=pm[:D, :])

            # ---- 3. B_rev per q-tile + DRAM bounce + single XBAR ----
            brd = dram.tile([1024, L], BF, tag="brd", name=f"brd{h % 2}")
            brd_base = brd[:].offset
            for qi in range(QT):
                w = (qi + 1) * 128
                br = brp.tile([128, w + 128], BF, tag=f"br{qi}")
                nc.vector.memset(br[:, w:w + 128], 0.0)
                e0 = S - 128 - qi * 128
                for c0 in range(0, w, 512):
                    cw = min(512, w - c0)
                    pm = ps_mm.tile([128, 512], F32, tag="mm")
                    nc.tensor.matmul(pm[:, :cw], vT[:, bass.ts(qi, 128)],
                                     c_et[:, e0 + c0:e0 + c0 + cw],
                                     start=True, stop=True)
                    nc.vector.tensor_copy(out=br[:, c0:c0 + cw],
                                          in_=pm[:, :cw])
                eng = nc.gpsimd if qi % 2 == 0 else nc.sync
                eng.dma_start(
                    out=bass.AP(tensor=brd[:].tensor,
                                offset=brd_base + qi * QSTRIDE,
                                ap=[[L, 128], [1, w + 128]]),
                    in_=br[:])
            st_all = stp.tile([128, QT, S], BF, tag="st")
            nc.sync.dma_start_transpose(
                out=st_all[:],
                in_=bass.AP(tensor=brd[:].tensor, offset=brd_base + 127,
                            ap=[[L - 1, 1024], [1, 1024]]))
            cur["st"] = st_all

            # ---- 4. normal-form vals with ones column: vN [128, QT, DA] ----
            vn = vnp.tile([128, QT, DA], BF, tag="vN")
            for g in range(2):
                pm = ps_mm.tile([128, 512], F32, tag="mm")
                for lt4 in range(4):
                    lt = g * 4 + lt4
                    nc.tensor.matmul(
                        pm[:, lt4 * DA:(lt4 + 1) * DA],
                        xin[:, 0, bass.ts(lt, 128)], c_wvn[:],
                        start=True, stop=True)
                nc.vector.tensor_copy(
                    out=vn[:, g * 4:(g + 1) * 4, :].rearrange(
                        "p a b -> p (a b)"),
                    in_=pm[:, :4 * DA])
            cur["vn"] = vn
            vsum = small.tile([D, 1], F32, tag="vsum")
            nc.vector.tensor_reduce(out=vsum[:], in_=vT[:],
                                    axis=mybir.AxisListType.X,
                                    op=mybir.AluOpType.add)

            # ---- 5. scoresT -> exp (strict upper j>i), diag masked ----
            et_tiles = []
            for lb in range(QT):
                wl = 128 * (lb + 1)
                et = expp.tile([128, wl], BF, tag=f"expT{lb}")
                for c0 in range(0, wl, 512):
                    cw = min(512, wl - c0)
                    pm = ps_mm.tile([128, 512], F32, tag="mm")
                    nc.tensor.matmul(pm[:, :cw],
                                     kT[:, bass.ts(lb, 128)],
                                     qT[:, c0:c0 + cw],
                                     start=True, stop=True)
                    nc.scalar.activation(
                        out=et[:, c0:c0 + cw], in_=pm[:, :cw],
                        func=mybir.ActivationFunctionType.Exp)
                nc.vector.tensor_mul(et[:, bass.ts(lb, 128)],
                                     et[:, bass.ts(lb, 128)], c_m01[:])
                et_tiles.append(et)

            # ---- 6. combine-mul of previous head (frees zp banks) ----
            if prev is not None:
                emit_combine_mul(prev)

            # ---- 7. zpT accumulation + last-row fix + 1/Z ----
            zp = [ps_zp0.tile([DA, 512], F32, tag="zp0", name="zp0"),
                  ps_zp1.tile([DA, 512], F32, tag="zp1", name="zp1")]
            cur["zp"] = zp
            # one start per PSUM bank (see zsT comment): zp0 starts at
            # (lb=0, qb=0), zp1 at its first touch (lb=4, qb=4).
            for lb in range(QT):
                for qb in range(lb + 1):
                    g, coff = qb // 4, (qb % 4) * 128
                    nc.tensor.matmul(
                        zp[g][:, coff:coff + 128],
                        vn[:, lb, :],
                        et_tiles[lb][:, bass.ts(qb, 128)],
                        start=(lb == 0 or (lb == 4 and qb == 4)),
                        stop=(lb == QT - 1 and qb in (3, 7)))
            # uniform last row: zpT[:, 1023] = [colsum(vals); S]
            nc.vector.tensor_copy(out=zp[1][0:D, 511:512], in_=vsum[:])
            nc.vector.memset(zp[1][D:DA, 511:512], float(S))
            rz = small.tile([1, S], F32, tag="rcpZ")
            for g in range(2):
                nc.vector.reciprocal(rz[:, bass.ts(g, 512)], zp[g][D:DA, :])
            # partition-broadcast of 1/Z needs linear addressing -> tiny DRAM
            # bounce (4KB out, 256KB back), then [D, S] broadcast read.
            rzd = dram.tile([1, S], F32, tag="rzd", name=f"rzd{h % 2}")
            nc.gpsimd.dma_start(out=rzd[:], in_=rz[:])
            binv = binvp.tile([D, S], F32, tag="binv")
            nc.gpsimd.dma_start(out=binv[:], in_=rzd[:].to_broadcast((D, S)))
            cur["binv"] = binv

            # ---- 8. previous head: zsT + combine ----
            if prev is not None:
                emit_zsT(prev)
                emit_combine_add(prev)
            prev = cur

        # drain pipeline: last head's zsT + combine
        emit_combine_mul(prev)
        emit_zsT(prev)
        emit_combine_add(prev)

        # ---- output projection: outT[m-chunk] = WoT-chunks @ zTc + bo ----
        for m in range(4):
            for sl in range(2):
                pm = ps_mm.tile([128, 512], F32, tag="mm")
                for kc in range(4):
                    nc.tensor.matmul(
                        pm[:], c_wo[:, kc, bass.ts(m, 128)],
                        zTc[kc][:, bass.ts(sl, 512)],
                        start=(kc == 0), stop=(kc == 3))
                ot = outp.tile([128, 512], BF, tag="ot")
                nc.scalar.add(out=ot[:], in_=pm[:], add=c_bo[:, m, :])
                nc.gpsimd.dma_start(
                    out=outT[bass.ts(m, 128), bass.ts(sl, 512)],
                    in_=ot[:])

    nc.compile()
    return nc


_nc_cache = {}


def _get_nc():
    if "nc" not in _nc_cache:
        _nc_cache["nc"] = build_nc()
    return _nc_cache["nc"]


def _prep_inputs(v, k, q, Wv, bv, Wk, bk, Wq, bq, E, Wo, bo):
    """Host-side layout prep (numpy only). Returns per-core input maps."""
    f = np.asarray
    v, k, q = f(v, np.float32), f(k, np.float32), f(q, np.float32)
    Wv, bv = f(Wv, np.float32), f(bv, np.float32)
    Wk, bk = f(Wk, np.float32), f(bk, np.float32)
    Wq, bq = f(Wq, np.float32), f(bq, np.float32)
    E, Wo, bo = f(E, np.float32), f(Wo, np.float32), f(bo, np.float32)

    scale = 1.0 / np.sqrt(np.float32(EMB))

    def prep_x(x):  # (N,S,EMB) -> (N,H,DA,S)
        xt = x.reshape(N, S, H, D).transpose(0, 2, 3, 1)
        ones = np.ones((N, H, 1, S), np.float32)
        return np.concatenate([xt, ones], axis=2)

    vkq = np.stack([prep_x(v), prep_x(k), prep_x(q)],
                   axis=3).astype(bf16)  # (N, H, DA, 3, S)

    def prep_w(W_, b_, s=1.0):
        return np.concatenate(
            [W_.T * s, b_[None, :] * s], 0).astype(bf16)

    wv, wk, wq = prep_w(Wv, bv), prep_w(Wk, bk), prep_w(Wq, bq, scale)
    wvn = np.zeros((DA, DA), np.float32)
    wvn[:D, :D] = Wv.T
    wvn[D, :D] = bv
    wvn[D, D] = 1.0
    wvn = wvn.astype(bf16)
    eta = np.ascontiguousarray(E[0].T).astype(bf16)          # [D, S]
    wot = np.ascontiguousarray(Wo.T).astype(bf16)            # [e, e_out]
    boc = np.ascontiguousarray(bo.reshape(4, 128, 1)).astype(np.float32)
    m01 = np.tril(np.ones((128, 128), np.float32), -1).astype(bf16)

    shared = {"WvTa": wv, "WkTa": wk, "WqTa": wq, "WvN": wvn, "ETa": eta,
              "WoT": wot, "boC": boc, "M01": m01}
    return [
        {"vkq": np.ascontiguousarray(vkq[n]), **shared}
        for n in range(N)
    ]


def kernel(v, k, q, Wv, bv, Wk, bk, Wq, bq, E, Wo, bo):
    in_maps = _prep_inputs(v, k, q, Wv, bv, Wk, bk, Wq, bq, E, Wo, bo)
    nc = _get_nc()
    res = run_bass_kernel_spmd(nc, in_maps, list(range(NCORES)))
    out = np.stack([np.asarray(res.results[n]["outT"]).astype(np.float32)
                    for n in range(N)])  # (N,EMB,S)
    return np.ascontiguousarray(out.transpose(0, 2, 1))
